# revision 18
# baseline (speedup 1.0000x reference)
"""Trainium2 Bass kernel for Mixtral-style attention (B=2, S=2048, 32 q / 8 kv heads, D=128).

Sharding: 2-way data parallel over batch x 4-way tensor parallel over heads
(8 cores). Each core computes QKV projection for its head shard, RoPE, causal
GQA attention, and a partial o_proj (row-sharded). Host sums the 4 partials
per batch element.

All heavy matmuls run in bf16 with fp32 PSUM accumulation; softmax runs in
fp32 on the scalar engine (exp folded with the 1/sqrt(d) scale, denominator
via the activation accumulator, normalization folded into the probs-transpose
matmul as a diagonal rhs).
"""

import os
import sys

import numpy as np

for _p in ("/opt/trn_rl_repo", "/root/.axon_site/_ro/trn_rl_repo"):
    if os.path.isdir(_p) and _p not in sys.path:
        sys.path.insert(0, _p)

import ml_dtypes  # noqa: E402

import concourse.bass as bass  # noqa: E402
import concourse.mybir as mybir  # noqa: E402
import concourse.tile as tile  # noqa: E402
from concourse import bacc, bass_utils  # noqa: E402

BF16 = ml_dtypes.bfloat16
F32 = mybir.dt.float32
BF = mybir.dt.bfloat16

B, S, HIDDEN = 2, 2048, 4096
NH, NKV, D = 32, 8, 128
TP, DP = 4, 2  # head-parallel x batch-parallel = 8 cores
QH = NH // TP  # 8 q heads per core
KH = NKV // TP  # 2 kv heads per core
NC_TILES = QH + 2 * KH  # 12 c-tiles of 128 per core (q..., k..., v...)
SC = 512  # s-chunk for phase A / attnT free dim
NSC = S // SC  # 4
NBLK = S // 128  # 16
ROPE_THETA = 10000.0
SM_SCALE = float(D) ** -0.5


def _emit(nc: bass.Bass):
    hT = nc.dram_tensor("hT", [128, HIDDEN // 128, S], BF, kind="ExternalInput")
    wq = nc.dram_tensor("wq", [NC_TILES, 128, 32 * 128], BF, kind="ExternalInput")
    wo = nc.dram_tensor("wo", [8, 128, 8 * 512], BF, kind="ExternalInput")
    cosT = nc.dram_tensor("cosT", [128, S], BF, kind="ExternalInput")
    sinT = nc.dram_tensor("sinT", [128, S], BF, kind="ExternalInput")
    triuD = nc.dram_tensor("triuD", [128, 128], BF, kind="ExternalInput")
    onesD = nc.dram_tensor("onesD", [1, 128], BF, kind="ExternalInput")
    onesCD = nc.dram_tensor("onesCD", [128, 1], BF, kind="ExternalInput")
    out = nc.dram_tensor("out", [S, HIDDEN], F32, kind="ExternalOutput")

    with tile.TileContext(nc) as tc:
        with (
            tc.tile_pool(name="const", bufs=1) as constp,
            tc.tile_pool(name="big", bufs=2) as bigp,
            tc.tile_pool(name="wt", bufs=3) as wtp,
            tc.tile_pool(name="pers", bufs=1) as pers,
            tc.tile_pool(name="rope", bufs=2) as ropep,
            tc.tile_pool(name="small", bufs=2) as smallp,
            tc.tile_pool(name="outp", bufs=2) as outp,
            tc.tile_pool(name="psum", bufs=2, space="PSUM") as psum,
            tc.tile_pool(name="psum_s", bufs=4, space="PSUM") as psum_s,
        ):
            cos_sb = constp.tile([128, S], BF, tag="cos")
            sin_sb = constp.tile([128, S], BF, tag="sin")
            triu = constp.tile([128, 128], BF, tag="triu")
            ones1 = constp.tile([1, 128], BF, tag="ones1")
            onesC = constp.tile([128, 1], BF, tag="onesC")
            nc.sync.dma_start(cos_sb, cosT[:])
            nc.sync.dma_start(sin_sb, sinT[:])
            nc.sync.dma_start(triu, triuD[:])
            nc.sync.dma_start(ones1, onesD[:])
            nc.sync.dma_start(onesC, onesCD[:])

            # persistent activations
            qT = pers.tile([128, QH, S], BF, tag="qT")  # [d, head, s]
            kT = pers.tile([128, KH, S], BF, tag="kT")
            vN = pers.tile([128, KH * NBLK, 128], BF, tag="vN")  # [sk, kv*blk, d]
            aT = pers.tile([128, QH, S], BF, tag="aT")  # [d, head, s]

            def rope_into(dst, ps, sc):
                # dst = ps * cos + rot(ps) * sin ; rot = [-x2, x1]
                rot = ropep.tile([128, SC], F32, tag="rot")
                nc.scalar.mul(rot[0:64, :], ps[64:128, :], -1.0)
                nc.scalar.copy(rot[64:128, :], ps[0:64, :])
                t2 = ropep.tile([128, SC], F32, tag="t2")
                cs = cos_sb[:, sc * SC : (sc + 1) * SC]
                sn = sin_sb[:, sc * SC : (sc + 1) * SC]
                nc.vector.tensor_mul(t2, ps, cs)
                nc.vector.tensor_mul(rot, rot, sn)
                nc.vector.tensor_add(dst, t2, rot)

            # ---- Phase A: QKV^T = w_shard^T @ hidden^T, RoPE, V transpose ----
            for sc in range(NSC):
                hTc = bigp.tile([128, 32, SC], BF, tag="bigslot")
                for hq in range(4):
                    nc.sync.dma_start(
                        hTc[:, hq * 8 : (hq + 1) * 8, :],
                        hT[:, hq * 8 : (hq + 1) * 8, sc * SC : (sc + 1) * SC],
                    )
                for c in range(NC_TILES):
                    wct = wtp.tile([128, 32 * 128], BF, tag="wt")
                    for hq in range(4):
                        nc.sync.dma_start(
                            wct[:, hq * 1024 : (hq + 1) * 1024],
                            wq[c, :, hq * 1024 : (hq + 1) * 1024],
                        )
                    ps = psum.tile([128, SC], F32, tag="mm512")
                    for ho in range(32):
                        nc.tensor.matmul(
                            ps,
                            wct[:, ho * 128 : (ho + 1) * 128],
                            hTc[:, ho, :],
                            start=(ho == 0),
                            stop=(ho == 31),
                        )
                    if c < QH:
                        rope_into(qT[:, c, sc * SC : (sc + 1) * SC], ps, sc)
                    elif c < QH + KH:
                        rope_into(kT[:, c - QH, sc * SC : (sc + 1) * SC], ps, sc)
                    else:
                        kv = c - QH - KH
                        vt = ropep.tile([128, SC], BF, tag="vt")
                        nc.scalar.copy(vt, ps)
                        for j in range(SC // 128):
                            blk = sc * 4 + j
                            nc.sync.dma_start(
                                vN[:, kv * NBLK + blk, :],
                                vt[:, j * 128 : (j + 1) * 128],
                                transpose=True,
                            )

            # ---- Phase B: causal GQA attention per head ----
            # slab[:, j, :] holds (unnormalized) probsT for sk-block j of the
            # current sq-chunk: ALL scores are computed directly transposed
            # (kT_blk^T @ qT_chunk) + exp from PSUM. Diagonal rows only cover
            # their causal sq columns; the diagonal 128x128 block gets a
            # transposed-tril (triu) mask applied post-exp. Softmax
            # denominator = ones^T @ slab rows on PE; normalization folded
            # into the attnT epilogue via a broadcast reciprocal row.
            def b_scores(h, m):
                kv = h // (QH // KH)
                slab = bigp.tile([128, NBLK, SC], BF, tag="bigslot")
                qm = qT[:, h, m * 512 : (m + 1) * 512]
                for j in range(4 * m + 4):
                    jj = j - 4 * m  # >= 0 for diagonal-region rows
                    c0 = max(0, jj) * 128
                    sps = psum_s.tile([128, 512], F32, tag="scores")
                    nc.tensor.matmul(
                        sps[:, : 512 - c0],
                        kT[:, kv, j * 128 : (j + 1) * 128],
                        qm[:, c0:],
                        start=True,
                        stop=True,
                    )
                    nc.scalar.activation(
                        slab[:, j, c0:],
                        sps[:, : 512 - c0],
                        mybir.ActivationFunctionType.Exp,
                        scale=SM_SCALE,
                    )
                    if jj >= 0:
                        blk = slab[:, j, c0 : c0 + 128]
                        nc.vector.tensor_mul(blk, blk, triu)
                return slab

            def b_denattn(h, m, slab):
                kv = h // (QH // KH)
                den = psum.tile([1, 512], F32, tag="mm512")
                aps = psum.tile([128, 512], F32, tag="attn")
                for j in range(4 * m):
                    sl = slab[:, j, :]
                    nc.tensor.matmul(
                        den, onesC, sl, start=(j == 0), stop=False,
                        skip_group_check=True,
                    )
                    nc.tensor.matmul(
                        aps, vN[:, kv * NBLK + j, :], sl, start=(j == 0),
                        stop=False, skip_group_check=True,
                    )
                for jj in range(4):
                    j = 4 * m + jj
                    cs = slice(jj * 128, 512)
                    sl = slab[:, j, cs]
                    first = m == 0 and jj == 0
                    nc.tensor.matmul(
                        den[:, cs], onesC, sl, start=first, stop=(jj == 3),
                        skip_group_check=True,
                    )
                    nc.tensor.matmul(
                        aps[:, cs], vN[:, kv * NBLK + j, :], sl, start=first,
                        stop=(jj == 3), skip_group_check=True,
                    )
                rrow = smallp.tile([1, 512], F32, tag="rr")
                nc.vector.reciprocal_approx_fast(rrow, den)
                rrow_bf = smallp.tile([1, 512], BF, tag="rrb")
                nc.vector.tensor_copy(rrow_bf, rrow)
                return aps, rrow_bf

            def b_epilogue(h, m, aps, rrow_bf):
                rps = psum_s.tile([128, 512], F32, tag="scores")
                nc.tensor.matmul(rps, ones1, rrow_bf, start=True, stop=True)
                rcp = smallp.tile([128, 512], BF, tag="rcp")
                nc.vector.tensor_copy(rcp, rps)
                nc.vector.tensor_mul(aT[:, h, m * 512 : (m + 1) * 512], aps, rcp)

            # 3-stage software pipeline over (head, chunk): scores(k) runs on
            # PE while ACT computes exps for k and PE consumes slab(k-1);
            # epilogue(k-2) trails so its DVE chain is off the critical path.
            seq = [(h, m) for h in range(QH) for m in range(NSC)]
            st1 = st2 = None  # (h, m, slab) / (h, m, aps, rrow_bf)
            for k, (h, m) in enumerate(seq):
                slab = b_scores(h, m)
                if st1 is not None:
                    ph, pm, pslab = st1
                    st2_new = (ph, pm) + b_denattn(ph, pm, pslab)
                    if st2 is not None:
                        b_epilogue(*st2)
                    st2 = st2_new
                st1 = (h, m, slab)
            ph, pm, pslab = st1
            st2_new = (ph, pm) + b_denattn(ph, pm, pslab)
            if st2 is not None:
                b_epilogue(*st2)
            b_epilogue(*st2_new)

            # ---- Phase C: partial o_proj = attnT^T @ w_o_shard ----
            for hc in range(8):
                wot = wtp.tile([128, 8 * 512], BF, tag="wt")
                for hq in range(4):
                    nc.sync.dma_start(
                        wot[:, hq * 1024 : (hq + 1) * 1024],
                        wo[hc, :, hq * 1024 : (hq + 1) * 1024],
                    )
                for st in range(NBLK):
                    ops = psum.tile([128, 512], F32, tag="mm512")
                    for cb in range(QH):
                        nc.tensor.matmul(
                            ops,
                            aT[:, cb, st * 128 : (st + 1) * 128],
                            wot[:, cb * 512 : (cb + 1) * 512],
                            start=(cb == 0),
                            stop=(cb == QH - 1),
                        )
                    ot = outp.tile([128, 512], F32, tag="ot")
                    nc.vector.tensor_copy(ot, ops)
                    nc.sync.dma_start(
                        out[st * 128 : (st + 1) * 128, hc * 512 : (hc + 1) * 512], ot
                    )

    return nc


_CACHE = {}


def build_program():
    if "nc" not in _CACHE:
        nc = bacc.Bacc()
        _emit(nc)
        nc.compile()
        _CACHE["nc"] = nc
    return _CACHE["nc"]


def host_inputs(positions, hidden_states, w_qkv, w_o):
    """Build the 8 per-core input maps (host-side shard + layout + bf16 cast)."""
    positions = np.asarray(positions)
    hidden_states = np.asarray(hidden_states, dtype=np.float32)
    w_qkv = np.asarray(w_qkv, dtype=np.float32)
    w_o = np.asarray(w_o, dtype=np.float32)

    inv_freq = 1.0 / (
        ROPE_THETA ** (np.arange(0, D, 2, dtype=np.float32) / D)
    )  # [64]
    trium = np.triu(np.ones((128, 128), dtype=np.float32)).astype(BF16)

    # per-batch tensors
    hTs, coss, sins = [], [], []
    for b in range(B):
        hT = (
            np.ascontiguousarray(hidden_states[b].T)  # [HIDDEN, S]
            .reshape(HIDDEN // 128, 128, S)
            .transpose(1, 0, 2)  # [128, ho, S]
        )
        hTs.append(np.ascontiguousarray(hT.astype(BF16)))
        ang = positions[b].astype(np.float32)[:, None] * inv_freq[None, :]  # [S,64]
        c = np.cos(ang).T  # [64, S]
        s = np.sin(ang).T
        coss.append(np.concatenate([c, c], axis=0).astype(BF16))
        sins.append(np.concatenate([s, s], axis=0).astype(BF16))

    in_maps = []
    for core in range(8):
        b, t = divmod(core, TP)
        qcols = w_qkv[:, t * QH * D : (t + 1) * QH * D]
        kcols = w_qkv[:, NH * D + t * KH * D : NH * D + (t + 1) * KH * D]
        vcols = w_qkv[:, (NH + NKV) * D + t * KH * D : (NH + NKV) * D + (t + 1) * KH * D]
        wshard = np.concatenate([qcols, kcols, vcols], axis=1)  # [4096, 1536]
        wq_t = (
            wshard.reshape(32, 128, NC_TILES, 128)
            .transpose(2, 1, 0, 3)  # [c, p, ho, m]
            .reshape(NC_TILES, 128, 32 * 128)
            .astype(BF16)
        )
        wo_shard = w_o[t * QH * D : (t + 1) * QH * D, :]  # [1024, 4096]
        wo_t = (
            wo_shard.reshape(QH, 128, 8, 512)
            .transpose(2, 1, 0, 3)  # [hc, p, co, n]
            .reshape(8, 128, 8 * 512)
            .astype(BF16)
        )
        in_maps.append(
            {
                "hT": hTs[b],
                "wq": np.ascontiguousarray(wq_t),
                "wo": np.ascontiguousarray(wo_t),
                "cosT": coss[b],
                "sinT": sins[b],
                "triuD": trium,
                "onesD": np.ones((1, 128), dtype=BF16),
                "onesCD": np.ones((128, 1), dtype=BF16),
            }
        )
    return in_maps


def gather_output(results):
    """Sum the 4 TP partials per batch -> [B, S, HIDDEN] fp32."""
    outs = []
    for b in range(B):
        acc = np.zeros((S, HIDDEN), dtype=np.float32)
        for t in range(TP):
            acc += results[b * TP + t]["out"]
        outs.append(acc)
    return np.stack(outs, axis=0)


def kernel(positions, hidden_states, w_qkv, w_o, trace=False):
    nc = build_program()
    in_maps = host_inputs(positions, hidden_states, w_qkv, w_o)
    res = bass_utils.run_bass_kernel_spmd(
        nc, in_maps, core_ids=list(range(8)), trace=trace
    )
    out = gather_output(res.results)
    if trace:
        kernel.last_exec_time_ns = res.exec_time_ns
        kernel.last_results = res
    return out
